# revision 54
# baseline (speedup 1.0000x reference)
"""Trainium2 Bass kernel for nn_EdgeDecoder (GNN edge decoder, 2 relations).

Strategy (data-parallel over edges, 8 NeuronCores):
  - Shard the 500k edges of each relation across 8 cores (62500/core), then
    split each core's shard into 2 sub-shards so compact tables fit int16.
  - Per (core, relation, sub-shard) the host co-designs the edge order and
    the compact table layout: edges where BOTH endpoints are first
    occurrences ("F" edges, ~80%) come first, and both tables store rows in
    first-occurrence order.  The F-block therefore reads table rows
    0,1,2,... strictly sequentially on BOTH sides.  The host additionally
    uploads a column-major ([dim, row]) copy of the first FCAP table rows,
    so the whole F-block is fetched with plain 2D HWDGE dma_starts (128
    partitions x 8KB lines, full line rate, zero gather descriptors, zero
    xbar transposes) landing directly in [dim, edge] layout.
  - The remaining "L" edges (~20%) use SWDGE dma_gather (edge-major) from
    the row-major tables, PE-transposed on device per 128-block.
    (dma_gather(transpose=True) and concurrent dma_start_transpose proved
    unreliable on HW - both corrupt under concurrency; avoided entirely.)
  - Compute per 512-edge chunk, emitted in chunk-pairs so matmuls sharing
    the same stationary weights run back-to-back:
      pre_h = W1u^T tu + W1v^T tv      (fp16 matmuls, f32 PSUM, 2 hid halves)
      y0 = relu(pre0 + b1_0)           (Scalar engine)
      y1 = relu(pre1 + b1_1)           (Vector engine, fused add+max)
      logit = w2_0^T y0 + w2_1^T y1    (PE reduce matmuls, DVE copy out)
    The W2 stage is software-pipelined one chunk-pair behind W1; b2 is
    added host-side during the final scatter.
  - Logits stay sharded; host scatters them back to original edge order.
"""
import sys

if "/opt/trn_rl_repo" not in sys.path:
    sys.path.insert(0, "/opt/trn_rl_repo")

import numpy as np

P = 128
D = 128
HID = 256
E = 500000
NCORES = 8
EPC = E // NCORES          # 62500 edges per core per relation
NSUB = 2
SUBL = EPC // NSUB         # 31250 edges per sub-shard
NREL = 2
CCH = 512                  # edges per compute chunk

FCAP = 25600               # padded F (streamed) edges per shard; 25 cc-pairs
FGROUPS = [(0, 4096), (4096, 4096), (8192, 4096), (12288, 4096),
           (16384, 4096), (20480, 4096), (24576, 1024)]
N1024 = False              # emit W1/W2 matmuls at N=1024 (pair-wide)
LCAP = 6656                # padded L (gathered) edges per shard; 13 ccs
LBLKS = [(0, 3584), (3584, 3072)]
OUTW = FCAP + LCAP         # device logits per shard

UTROWS = 27136             # compact user-table rows (max uniq_u 26933)
VTROWS = 29184             # compact item-table rows (max uniq_v 29052)

_PROGRAM_CACHE = {}
LAST_RESULTS = None
DEBUG_BARRIER = False


def _build_program():
    import concourse.bacc as bacc
    import concourse.mybir as mybir
    from concourse.tile import TileContext

    f16, f32, i16 = mybir.dt.float16, mybir.dt.float32, mybir.dt.int16
    Relu = mybir.ActivationFunctionType.Relu
    Ident = mybir.ActivationFunctionType.Identity
    ADD, MAX = mybir.AluOpType.add, mybir.AluOpType.max

    nc = bacc.Bacc("TRN2", target_bir_lowering=False, debug=False,
                   num_swdge_queues=4)

    tabs, stabs, idxs_d, outs = {}, {}, {}, {}
    for r in range(NREL):
        for s in range(NSUB):
            tabs[("u", r, s)] = nc.dram_tensor(
                f"ut{r}_{s}", [UTROWS, D], f16, kind="ExternalInput")
            tabs[("v", r, s)] = nc.dram_tensor(
                f"vt{r}_{s}", [VTROWS, D], f16, kind="ExternalInput")
            stabs[("u", r, s)] = nc.dram_tensor(
                f"ust{r}_{s}", [P, FCAP], f16, kind="ExternalInput")
            stabs[("v", r, s)] = nc.dram_tensor(
                f"vst{r}_{s}", [P, FCAP], f16, kind="ExternalInput")
            for b, (boff, blen) in enumerate(LBLKS):
                idxs_d[("u", r, s, b)] = nc.dram_tensor(
                    f"ui{r}_{s}_{b}", [P, blen // 16], i16, kind="ExternalInput")
                idxs_d[("v", r, s, b)] = nc.dram_tensor(
                    f"vi{r}_{s}_{b}", [P, blen // 16], i16, kind="ExternalInput")
        outs[r] = nc.dram_tensor(f"o{r}", [NSUB, OUTW], f32,
                                 kind="ExternalOutput")
    w1u_d = [nc.dram_tensor(f"w1u{r}", [D, HID], f16, kind="ExternalInput")
             for r in range(NREL)]
    w1v_d = [nc.dram_tensor(f"w1v{r}", [D, HID], f16, kind="ExternalInput")
             for r in range(NREL)]
    w2_d = [nc.dram_tensor(f"w2{r}", [P, 2], f16, kind="ExternalInput")
            for r in range(NREL)]
    b1_d = [nc.dram_tensor(f"b1{r}", [P, 2], f32, kind="ExternalInput")
            for r in range(NREL)]
    id_d = nc.dram_tensor("ident", [P, P], f16, kind="ExternalInput")

    with TileContext(nc) as tc:
        with tc.tile_pool(name="sbw", bufs=1) as sbw, \
             tc.tile_pool(name="sbs", bufs=3) as sbs, \
             tc.tile_pool(name="sbg", bufs=2) as sbg, \
             tc.tile_pool(name="sbi", bufs=2) as sbi, \
             tc.tile_pool(name="sbh", bufs=3) as sbh, \
             tc.tile_pool(name="sblog", bufs=2) as sblog, \
             tc.tile_pool(name="sbt", bufs=3) as sbt, \
             tc.tile_pool(name="ph", bufs=2, space="PSUM") as ph, \
             tc.tile_pool(name="pt", bufs=2, space="PSUM") as pt, \
             tc.tile_pool(name="pl", bufs=1, space="PSUM") as pl:

            w1u_t, w1v_t, w2_t, b1_t = [], [], [], []
            for r in range(NREL):
                t = sbw.tile([D, HID], f16, tag=f"w1u{r}")
                nc.sync.dma_start(out=t[:], in_=w1u_d[r].ap()[:])
                w1u_t.append(t)
                t = sbw.tile([D, HID], f16, tag=f"w1v{r}")
                nc.sync.dma_start(out=t[:], in_=w1v_d[r].ap()[:])
                w1v_t.append(t)
                t = sbw.tile([P, 2], f16, tag=f"w2{r}")
                nc.sync.dma_start(out=t[:], in_=w2_d[r].ap()[:])
                w2_t.append(t)
                t = sbw.tile([P, 2], f32, tag=f"b1{r}")
                nc.sync.dma_start(out=t[:], in_=b1_d[r].ap()[:])
                b1_t.append(t)
            ident = sbw.tile([P, P], f16, tag="ident")
            nc.sync.dma_start(out=ident[:], in_=id_d.ap()[:])

            # pair-wide software pipeline: the W2 reduce of pair i-2 is
            # emitted at pair i, hiding the ACT/DVE relu latency of pair i-1
            PW = 2 * CCH
            pend = []

            def mm_w1(out_ap, lhsT, rhs_ap, w, start, stop):
                if N1024 or w <= CCH:
                    nc.tensor.matmul(out=out_ap[:, :w], lhsT=lhsT,
                                     rhs=rhs_ap, start=start, stop=stop,
                                     skip_group_check=True)
                else:
                    nc.tensor.matmul(out=out_ap[:, :CCH], lhsT=lhsT,
                                     rhs=rhs_ap[:, :CCH], start=start,
                                     stop=stop, skip_group_check=True)
                    nc.tensor.matmul(out=out_ap[:, CCH:w], lhsT=lhsT,
                                     rhs=rhs_ap[:, CCH:w], start=start,
                                     stop=stop, skip_group_check=True)

            def flush_pend(force=False):
                while pend and (force or len(pend) >= 2):
                    y0, y1, log_ap, w, r, gd = pend.pop(0)
                    pl_t = pl.tile([1, PW], f32, tag="pl", name="pl_t")
                    mm_w1(pl_t, w2_t[r][:, 0:1], y0[:, :w], w, True, False)
                    mm_w1(pl_t, w2_t[r][:, 1:2], y1[:, :w], w, False, True)
                    nc.scalar.activation(out=log_ap, in_=pl_t[:, :w],
                                         func=Ident, bias=0.0)
                    if gd is not None:
                        out_ap, log_t = gd
                        nc.sync.dma_start(out=out_ap, in_=log_t[:])

            def emit_pair(r, tu, tv, log_ap, w, gd):
                flush_pend()
                p0 = ph.tile([P, PW], f32, tag="ph", name="p0")
                p1 = ph.tile([P, PW], f32, tag="ph", name="p1")
                mm_w1(p0, w1u_t[r][:, 0:P], tu, w, True, False)
                mm_w1(p0, w1v_t[r][:, 0:P], tv, w, False, True)
                mm_w1(p1, w1u_t[r][:, P:2 * P], tu, w, True, False)
                mm_w1(p1, w1v_t[r][:, P:2 * P], tv, w, False, True)
                y0 = sbh.tile([P, PW], f16, tag="y0")
                nc.scalar.activation(out=y0[:, :w], in_=p0[:, :w], func=Relu,
                                     bias=b1_t[r][:, 0:1])
                y1 = sbh.tile([P, PW], f16, tag="y1")
                nc.vector.tensor_scalar(out=y1[:, :w], in0=p1[:, :w],
                                        scalar1=b1_t[r][:, 1:2],
                                        scalar2=0.0, op0=ADD, op1=MAX)
                pend.append((y0, y1, log_ap, w, r, gd))

            q = 0
            for r in range(NREL):
                for s in range(NSUB):
                    # kick the L gathers early: SWDGE desc-gen overlaps the
                    # entire F phase
                    gts = {}
                    for side in ("u", "v"):
                        for b, (boff, blen) in enumerate(LBLKS):
                            it = sbi.tile([P, blen // 16], i16,
                                          tag=f"i{side}{b}")
                            nc.sync.dma_start(
                                out=it[:], in_=idxs_d[(side, r, s, b)].ap()[:])
                            gt = sbg.tile([P, blen // P, D], f16,
                                          tag=f"g{side}{b}")
                            nc.gpsimd.dma_gather(
                                gt[:], tabs[(side, r, s)].ap()[:], it[:],
                                blen, blen, D,
                                single_packet=False, queue_num=q % 4)
                            q += 1
                            gts[(side, b)] = gt

                    # F phase: column-major tables stream as plain 2D loads
                    for goff, glen in FGROUPS:
                        su = sbs.tile([P, glen], f16, tag="su")
                        nc.sync.dma_start(
                            out=su[:],
                            in_=stabs[("u", r, s)].ap()[:, goff:goff + glen])
                        sv = sbs.tile([P, glen], f16, tag="sv")
                        nc.scalar.dma_start(
                            out=sv[:],
                            in_=stabs[("v", r, s)].ap()[:, goff:goff + glen])
                        log_t = sblog.tile([1, glen], f32, tag="log")
                        npair_g = glen // PW
                        for c in range(npair_g):
                            off = c * PW
                            gd = None
                            if c == npair_g - 1:
                                gd = (outs[r].ap()[s:s + 1,
                                                   goff:goff + glen], log_t)
                            emit_pair(r, su[:, off:off + PW],
                                      sv[:, off:off + PW],
                                      log_t[:, off:off + PW], PW, gd)
                        if DEBUG_BARRIER:
                            flush_pend(force=True)
                            tc.strict_bb_all_engine_barrier()

                    # L phase: gathered tiles (edge-major), PE-transposed
                    for b, (boff, blen) in enumerate(LBLKS):
                        gu, gv = gts[("u", b)], gts[("v", b)]
                        log_t = sblog.tile([1, blen], f32, tag="log")
                        c = 0
                        while c * CCH < blen:
                            w = min(PW, blen - c * CCH)
                            ptu = pt.tile([P, PW], f16, tag="pt", name="ptu")
                            for j in range(w // P):
                                nc.tensor.transpose(
                                    out=ptu[:, j * P:(j + 1) * P],
                                    in_=gu[:, c * (CCH // P) + j, :],
                                    identity=ident[:])
                            tu = sbt.tile([P, PW], f16, tag="tu")
                            nc.vector.tensor_copy(out=tu[:, :w],
                                                  in_=ptu[:, :w])
                            ptv = pt.tile([P, PW], f16, tag="pt", name="ptv")
                            for j in range(w // P):
                                nc.tensor.transpose(
                                    out=ptv[:, j * P:(j + 1) * P],
                                    in_=gv[:, c * (CCH // P) + j, :],
                                    identity=ident[:])
                            tv = sbt.tile([P, PW], f16, tag="tv")
                            nc.vector.tensor_copy(out=tv[:, :w],
                                                  in_=ptv[:, :w])
                            gd = None
                            if c * CCH + w == blen:
                                o0 = FCAP + boff
                                gd = (outs[r].ap()[s:s + 1, o0:o0 + blen],
                                      log_t)
                            emit_pair(r, tu[:, :w], tv[:, :w],
                                      log_t[:, c * CCH:c * CCH + w], w, gd)
                            c += 2
                        if DEBUG_BARRIER:
                            flush_pend(force=True)
                            tc.strict_bb_all_engine_barrier()
            flush_pend(force=True)
    nc.compile()
    return nc


def _wrap16_row(idx16):
    """[n] int16 -> [128, n//16] (16-wrap, replicated to 8 core groups)."""
    a = idx16.reshape(-1, 16).T
    return np.tile(a, (8, 1)).copy()


def _first_occ_mask(arr):
    m = np.zeros(len(arr), bool)
    m[np.unique(arr, return_index=True)[1]] = True
    return m


def _ordered_unique_fresh(vals, pos):
    """Unique values of `vals` with pos[v] < 0, in order of appearance."""
    fresh = vals[pos[vals] < 0]
    uvals, first = np.unique(fresh, return_index=True)
    return uvals[np.argsort(first)]


def _prep(user_embed, item_embed, u_clicks, v_clicks, u_buys, v_buys,
          W1_clicks, b1_clicks, W2_clicks, b2_clicks,
          W1_buys, b1_buys, W2_buys, b2_buys):
    NUM_USERS = user_embed.shape[0]
    NUM_ITEMS = item_embed.shape[0]
    user16 = np.asarray(user_embed, np.float32).astype(np.float16)
    item16 = np.asarray(item_embed, np.float32).astype(np.float16)
    rels = [
        (np.asarray(u_clicks, np.int64), np.asarray(v_clicks, np.int64),
         np.asarray(W1_clicks, np.float32), np.asarray(b1_clicks, np.float32),
         np.asarray(W2_clicks, np.float32), np.asarray(b2_clicks, np.float32)),
        (np.asarray(u_buys, np.int64), np.asarray(v_buys, np.int64),
         np.asarray(W1_buys, np.float32), np.asarray(b1_buys, np.float32),
         np.asarray(W2_buys, np.float32), np.asarray(b2_buys, np.float32)),
    ]

    in_maps, scat = [], {}
    pos_u = np.empty(NUM_USERS, np.int64)
    pos_v = np.empty(NUM_ITEMS, np.int64)
    for k in range(NCORES):
        m = {"ident": np.eye(P, dtype=np.float16)}
        for r in range(NREL):
            u_all, v_all, W1, b1, W2, b2 = rels[r]
            m[f"w1u{r}"] = W1[:D].astype(np.float16)
            m[f"w1v{r}"] = W1[D:].astype(np.float16)
            m[f"w2{r}"] = W2.reshape(2, P).T.astype(np.float16).copy()
            m[f"b1{r}"] = b1.reshape(2, P).T.astype(np.float32).copy()
            for s in range(NSUB):
                lo = k * EPC + s * SUBL
                u = u_all[lo:lo + SUBL]
                v = v_all[lo:lo + SUBL]
                F = _first_occ_mask(u) & _first_occ_mask(v)
                Fidx = np.where(F)[0]
                Lidx = np.where(~F)[0]
                nF, nL = len(Fidx), len(Lidx)
                if nF > FCAP or nL > LCAP:
                    raise RuntimeError(
                        f"shard ({k},{r},{s}): nF={nF} nL={nL} exceed "
                        f"FCAP={FCAP}/LCAP={LCAP}")
                # tables in first-occurrence order: F-block rows first
                pos_u.fill(-1)
                pos_u[u[Fidx]] = np.arange(nF)
                fresh_u = _ordered_unique_fresh(u[Lidx], pos_u)
                pos_u[fresh_u] = nF + np.arange(len(fresh_u))
                utab_ids = np.concatenate([u[Fidx], fresh_u])
                pos_v.fill(-1)
                pos_v[v[Fidx]] = np.arange(nF)
                fresh_v = _ordered_unique_fresh(v[Lidx], pos_v)
                pos_v[fresh_v] = nF + np.arange(len(fresh_v))
                vtab_ids = np.concatenate([v[Fidx], fresh_v])
                if len(utab_ids) > UTROWS or len(vtab_ids) > VTROWS:
                    raise RuntimeError(
                        f"shard ({k},{r},{s}): table overflow "
                        f"{len(utab_ids)}/{len(vtab_ids)}")
                tab = np.zeros((UTROWS, D), np.float16)
                tab[:len(utab_ids)] = user16[utab_ids]
                m[f"ut{r}_{s}"] = tab
                m[f"ust{r}_{s}"] = np.ascontiguousarray(tab[:FCAP].T)
                tab = np.zeros((VTROWS, D), np.float16)
                tab[:len(vtab_ids)] = item16[vtab_ids]
                m[f"vt{r}_{s}"] = tab
                m[f"vst{r}_{s}"] = np.ascontiguousarray(tab[:FCAP].T)
                # L gather indices, padded with 0
                lu = np.zeros(LCAP, np.int64)
                lv = np.zeros(LCAP, np.int64)
                lu[:nL] = pos_u[u[Lidx]]
                lv[:nL] = pos_v[v[Lidx]]
                for b, (boff, blen) in enumerate(LBLKS):
                    m[f"ui{r}_{s}_{b}"] = _wrap16_row(
                        lu[boff:boff + blen].astype(np.int16))
                    m[f"vi{r}_{s}_{b}"] = _wrap16_row(
                        lv[boff:boff + blen].astype(np.int16))
                ood = np.full(OUTW, -1, np.int64)
                ood[:nF] = Fidx
                ood[FCAP:FCAP + nL] = Lidx
                scat[(k, r, s)] = ood
        in_maps.append(m)
    b2s = [float(np.asarray(b2_clicks).reshape(-1)[0]),
           float(np.asarray(b2_buys).reshape(-1)[0])]
    return in_maps, scat, b2s


def make_in_maps(np_inputs):
    """For external harnesses: per-core input maps for the cached program."""
    return _prep(**np_inputs)[0]


def kernel(**inputs):
    global LAST_RESULTS
    from concourse import bass_utils

    in_maps, scat, b2s = _prep(**inputs)

    if "prog" not in _PROGRAM_CACHE:
        _PROGRAM_CACHE["prog"] = _build_program()
    nc = _PROGRAM_CACHE["prog"]

    res = bass_utils.run_bass_kernel_spmd(nc, in_maps,
                                          core_ids=list(range(NCORES)))
    LAST_RESULTS = res

    outs = []
    for r in range(NREL):
        full = np.empty(E, np.float32)
        for k in range(NCORES):
            o = res.results[k][f"o{r}"]          # [NSUB, OUTW]
            for s in range(NSUB):
                lo = k * EPC + s * SUBL
                ood = scat[(k, r, s)]
                valid = ood >= 0
                full[lo + ood[valid]] = o[s][valid]
        if b2s[r] != 0.0:
            full += b2s[r]
        outs.append(full)
    return outs[0], outs[1]
